# revision 1
# baseline (speedup 1.0000x reference)
"""Correlation (9x9 displacement) kernel for Trainium2.

out[b,c,i,j,y,x] = leaky_relu(ref[b,c,y,x] * tgt[b,c, y+j-4, x+i-4], 0.1)
with zero padding outside the target image bounds.

Sharding: the 256 (b,c) images are split 32-per-core across 8 NeuronCores
(pure data parallel, no collectives).

Per-core layout: partition p = yb*32 + n  (yb = row-block 0..3, n = image
0..31).  Each partition stores a halo tile of the target: 24 rows x 136 cols
(its 16-row block plus +-4 halo rows, W plus +-4 pad cols, zeros outside the
image).  Every displacement (i,j) then becomes the plain in-bounds slice
tgt[:, j:j+16, i:i+128], and out-of-bounds zeros compute themselves via
leaky(ref*0) == 0.  The halo construction happens on the host so each core
issues exactly two flat input DMAs.
"""

import numpy as np

import concourse.bacc as bacc
import concourse.bass as bass
import concourse.mybir as mybir
from concourse import bass_utils
from concourse.tile import TileContext

B, C, H, W = 4, 64, 64, 128
MD = 4
D = 2 * MD + 1  # 9
N_CORES = 8
IMGS = B * C  # 256
IPC = IMGS // N_CORES  # 32 images per core
YB = 4  # row blocks per image
BH = H // YB  # 16 rows per block
HALO_H = BH + 2 * MD  # 24
HALO_W = W + 2 * MD  # 136
F32 = mybir.dt.float32


def _build(
    jg: int = 1,
    mul_bufs: int = 6,
    out_bufs: int = 6,
    big_out: bool = False,
    skip_zeros: bool = True,
    gp_js: int = 0,
    act_prefetch: bool = False,
    split_in: bool = True,
) -> bass.Bass:
    nc = bacc.Bacc(trn_type="TRN2")
    ref_d = nc.dram_tensor("ref", [128, BH, W], F32, kind="ExternalInput")
    tgt_d = nc.dram_tensor("tgt", [128, HALO_H, HALO_W], F32, kind="ExternalInput")
    # Partition-major output: [p = yb*32+n, i, j, y_lo, x].  Keeps the store
    # DMA at 3 AP dims with 72KB-contiguous per-partition runs; the host
    # untangles (yb, n) during unsharding.
    out_d = nc.dram_tensor("out", [128, D, D, BH, W], F32, kind="ExternalOutput")

    with TileContext(nc) as tc:
        with (
            tc.tile_pool(name="const", bufs=1) as cpool,
            tc.tile_pool(name="mul", bufs=mul_bufs) as mpool,
            tc.tile_pool(name="outp", bufs=out_bufs) as opool,
        ):
            tgt_t = cpool.tile([128, HALO_H, HALO_W], F32)
            ref_t = cpool.tile([128, BH, W], F32)
            if act_prefetch:
                # Touch the Prelu table set before any data arrives so the
                # ~2.7us ACT_TABLE_LOAD overlaps the input DMAs.
                warm = cpool.tile([128, 1], F32)
                nc.vector.memset(warm[:], 0.0)
                nc.scalar.activation(
                    out=warm[:],
                    in_=warm[:],
                    func=mybir.ActivationFunctionType.Prelu,
                    alpha=0.1,
                )
            nc.sync.dma_start(out=ref_t[:], in_=ref_d[:])
            if split_in:
                nc.sync.dma_start(out=tgt_t[:, :BH], in_=tgt_d[:, :BH])
                nc.sync.dma_start(out=tgt_t[:, BH:], in_=tgt_d[:, BH:])
            else:
                nc.sync.dma_start(out=tgt_t[:], in_=tgt_d[:])
            for i in range(D):
                ot_big = None
                if big_out:
                    ot_big = opool.tile([128, D, BH, W], F32, name="otb", tag="otb")
                for jg_i in range(D // jg):
                    mt = mpool.tile([128, jg, BH, W], F32)
                    for jj in range(jg):
                        j = jg_i * jg + jj
                        # gp_js > 0 moves the first j's of each i to GPSIMD
                        # (measured slower in the cost model; default 0).
                        eng = nc.gpsimd if j < gp_js else nc.vector
                        eng.tensor_tensor(
                            out=mt[:, jj],
                            in0=ref_t[:],
                            in1=tgt_t[:, j : j + BH, i : i + W],
                            op=mybir.AluOpType.mult,
                        )
                    if big_out:
                        ot = ot_big[:, jg_i * jg : (jg_i + 1) * jg]
                    else:
                        ot_t = opool.tile([128, jg, BH, W], F32, name="ot", tag="ot")
                        ot = ot_t[:]
                    nc.scalar.activation(
                        out=ot,
                        in_=mt[:],
                        func=mybir.ActivationFunctionType.Prelu,
                        alpha=0.1,
                    )
                    if not big_out:
                        j0 = jg_i * jg
                        dj = j0 - MD
                        if skip_zeros and jg == 1 and dj != 0:
                            # Rows with y+dj out of [0,H) are structural zeros;
                            # the output buffer is pre-zeroed, so skip writing
                            # them.  They live in one partition block (yb=0
                            # for dj<0, yb=3 for dj>0), so the store splits
                            # into two contiguous DMAs.
                            if dj < 0:
                                nc.sync.dma_start(
                                    out=out_d[IPC:, i, j0], in_=ot[IPC:, 0]
                                )
                                nc.sync.dma_start(
                                    out=out_d[:IPC, i, j0, -dj:],
                                    in_=ot[:IPC, 0, -dj:],
                                )
                            else:
                                nc.sync.dma_start(
                                    out=out_d[: 3 * IPC, i, j0], in_=ot[: 3 * IPC, 0]
                                )
                                nc.sync.dma_start(
                                    out=out_d[3 * IPC :, i, j0, : BH - dj],
                                    in_=ot[3 * IPC :, 0, : BH - dj],
                                )
                        else:
                            nc.sync.dma_start(
                                out=out_d[:, i, j0 : j0 + jg],
                                in_=ot,
                            )
                if big_out:
                    nc.sync.dma_start(out=out_d[:, i], in_=ot_big[:])
    nc.finalize()
    return nc


_cached_nc = None
_last_results = None


def _prep_inputs(ref: np.ndarray, tgt: np.ndarray):
    """ref/tgt: (256, 64, 128) f32 -> per-core blocked/halo'd arrays.

    Returns ref_blocked (8, 128, 16, 128) and tgt_halo (8, 128, 24, 136),
    partition p = yb*32 + n.
    """
    # ref: (8 cores, 32 n, 4 yb, 16 y, 128 x) -> (8, yb, n, y, x)
    ref_b = ref.reshape(N_CORES, IPC, YB, BH, W).transpose(0, 2, 1, 3, 4)
    ref_b = np.ascontiguousarray(ref_b).reshape(N_CORES, 128, BH, W)

    tp = np.zeros((IMGS, H + 2 * MD, HALO_W), dtype=np.float32)
    tp[:, MD : MD + H, MD : MD + W] = tgt
    # overlapping 24-row windows starting at yb*16
    idx = (BH * np.arange(YB))[:, None] + np.arange(HALO_H)[None, :]
    halo = tp[:, idx, :]  # (256, 4, 24, 136)
    halo = halo.reshape(N_CORES, IPC, YB, HALO_H, HALO_W).transpose(0, 2, 1, 3, 4)
    halo = np.ascontiguousarray(halo).reshape(N_CORES, 128, HALO_H, HALO_W)
    return ref_b, halo


def kernel(refimg_fea: np.ndarray, targetimg_fea: np.ndarray) -> np.ndarray:
    global _cached_nc, _last_results
    ref = np.asarray(refimg_fea, dtype=np.float32).reshape(IMGS, H, W)
    tgt = np.asarray(targetimg_fea, dtype=np.float32).reshape(IMGS, H, W)
    ref_b, tgt_h = _prep_inputs(ref, tgt)
    if _cached_nc is None:
        _cached_nc = _build()
    nc = _cached_nc
    in_maps = [{"ref": ref_b[k], "tgt": tgt_h[k]} for k in range(N_CORES)]
    res = bass_utils.run_bass_kernel_spmd(nc, in_maps, core_ids=list(range(N_CORES)))
    _last_results = res
    # Per-core output is [yb*32+n, i, j, y_lo, x]; reassemble to
    # [n, i, j, (yb y_lo), x] per core, then stack cores along n.
    parts = []
    for r in res.results:
        o = r["out"].reshape(YB, IPC, D, D, BH, W)
        parts.append(o.transpose(1, 2, 3, 0, 4, 5).reshape(IPC, D, D, H, W))
    out = np.concatenate(parts, axis=0)
    return out.reshape(B, C, D, D, H, W)



# revision 25
# speedup vs baseline: 1.8219x; 1.8219x over previous
"""Correlation (9x9 displacement) kernel for Trainium2.

out[b,c,i,j,y,x] = leaky_relu(ref[b,c,y,x] * tgt[b,c, y+j-4, x+i-4], 0.1)
with zero padding outside the target image bounds.

Sharding: the 256 (b,c) images are split 32-per-core across 8 NeuronCores
(pure data parallel, no collectives).

Per-core layout: partition p = yb*32 + n  (yb = row-block 0..3, n = image
0..31).  Each partition stores a halo tile of the target: 24 rows x 136 cols
(its 16-row block plus +-4 halo rows, W plus +-4 pad cols, zeros outside the
image).  Every displacement (i,j) then becomes the plain in-bounds slice
tgt[:, j:j+16, i:i+128], and out-of-bounds zeros compute themselves via
leaky(tgt_pad*0) == 0.

The whole pipeline runs in bf16 (the correctness gate is rel_err < 2e-2;
the bf16 rounding chain stays near ~1e-2 worst case / ~4e-3 typical):
 - halves the output-store DMA bytes, which is the hard roofline here
   (the cost model serializes all DMA transfers at 360 B/ns per core), and
 - enables the DVE 2x/4x perf modes for the elementwise work.

Per i-column (fixed width offset i), one DVE tensor_tensor computes all 9
j-shifts at once via a 4D overlapping-window AP on the halo tile (the j and
y dims share the row stride) with a stride-0 broadcast AP for ref.  The
leaky is split across engines to stay under the DMA roofline:
  j=0..5 -> ACT Prelu (in-place),
  j=6..8 -> DVE tensor_scalar (0.1*m) + Pool tensor_tensor max(m, 0.1m),
using leaky(v) == max(v, 0.1*v) for slope 0.1.
Column i=0 is issued fine-grained (1/2/3-j chunks) so the first store fires
~8us in instead of ~25us (pipeline fill).

Structural zero rows (y+j-4 out of [0,64)) are skipped on the store; the
DRAM output buffer is pre-zeroed, and the zeros live in one 32-partition
block (yb=0 for j<4, yb=3 for j>4), so each such store splits in two.
"""

import ml_dtypes
import numpy as np

import concourse.bacc as bacc
import concourse.mybir as mybir
from concourse import bass_utils
from concourse.ap import AP
from concourse.tile import TileContext

B, C, H, W = 4, 64, 64, 128
MD = 4
D = 2 * MD + 1  # 9
N_CORES = 8
IMGS = B * C  # 256
IPC = IMGS // N_CORES  # 32 images per core
YB = 4  # row blocks per image
BH = H // YB  # 16 rows per block
HALO_H = BH + 2 * MD  # 24
HALO_W = W + 2 * MD  # 136
BF16 = mybir.dt.bfloat16
NP_BF16 = ml_dtypes.bfloat16


# Per-column op plan for columns 1..8 (column 0 is hand-scheduled fine-
# grained warmup).  Each entry: ("m", j0, j1) DVE multiply chunk,
# ("A", j0, j1) ACT Prelu + stores, ("V", j0, j1) DVE scale+max + stores.
_STD = [
    ("m", 0, 4), ("A", 0, 2), ("A", 2, 4), ("m", 7, 9), ("t", 7, 9), ("d",),
    ("x", 7, 9), ("m", 4, 7), ("A", 4, 6), ("A", 6, 7),
]
_K1 = [
    ("m", 0, 4), ("A", 0, 2), ("A", 2, 4), ("m", 8, 9), ("t", 8, 9), ("d",),
    ("x", 8, 9), ("m", 4, 8), ("A", 4, 6), ("A", 6, 8),
]
_C1 = [
    ("m", 0, 2), ("A", 0, 2), ("m", 2, 4), ("A", 2, 4),
    ("m", 7, 9), ("t", 7, 9), ("d",), ("x", 7, 9),
    ("m", 4, 7), ("A", 4, 6), ("A", 6, 7),
]
_C0 = [
    ("m", 0, 1, 0, 8), ("A", 0, 1, 0, 8), ("m", 0, 1, 8, 16), ("A", 0, 1, 8, 16),
    ("m", 1, 2), ("A", 1, 2), ("m", 7, 8), ("V", 7, 8),
    ("m", 2, 3), ("A", 2, 3), ("m", 8, 9), ("t2", 8, 9), ("x2", 8, 9),
    ("m", 3, 4), ("A", 3, 4), ("m", 4, 5), ("A", 4, 5), ("m", 5, 7), ("A", 5, 7),
]
PLAN = {0: _C0, 1: _C1, 2: _STD, 3: _STD, 4: _K1, 5: _STD, 6: _STD, 7: _STD, 8: _STD}


def _build() -> "bacc.Bacc":
    nc = bacc.Bacc(trn_type="TRN2")
    ref_d = nc.dram_tensor("ref", [128, BH, W], BF16, kind="ExternalInput")
    tgt_d = nc.dram_tensor("tgt", [128, HALO_H, HALO_W], BF16, kind="ExternalInput")
    # Partition-major output: [p = yb*32 + n, i, j, y_lo, x]; the host
    # untangles (yb, n) during unsharding.
    out_d = nc.dram_tensor("out", [128, D, D, BH, W], BF16, kind="ExternalOutput")

    with TileContext(nc) as tc:
        with (
            tc.tile_pool(name="const", bufs=1) as cpool,
            tc.tile_pool(name="m", bufs=3) as mpool,
            tc.tile_pool(name="am", bufs=3) as apool,
        ):
            tgt_t = cpool.tile([128, HALO_H, HALO_W], BF16)
            ref_t = cpool.tile([128, BH, W], BF16)
            # Fine-grained loads so the first compute chunk (j=0, rows 0:8)
            # only waits for the first two transfers.  (No finer: HWDGE
            # costs 625ns per DMA, so below ~500B/partition the input
            # loading becomes descriptor-generation-bound.)
            HB = BH // 2  # 8
            nc.sync.dma_start(out=tgt_t[:, :HB], in_=tgt_d[:, :HB])
            nc.sync.dma_start(out=ref_t[:, :HB], in_=ref_d[:, :HB])
            nc.sync.dma_start(out=ref_t[:, HB:], in_=ref_d[:, HB:])
            nc.sync.dma_start(out=tgt_t[:, HB:BH], in_=tgt_d[:, HB:BH])
            nc.sync.dma_start(out=tgt_t[:, BH:], in_=tgt_d[:, BH:])

            ta = tgt_t[:]

            def win(i: int, j0: int, j1: int) -> AP:
                # [p, j, y, x] overlapping-window view of the halo tile:
                # j and y share the row stride.
                return AP(
                    tensor=ta.tensor,
                    offset=ta.offset + j0 * HALO_W + i,
                    ap=[list(ta.ap[0]), [HALO_W, j1 - j0], [HALO_W, BH], [1, W]],
                )

            def rbc(nj: int) -> AP:
                return ref_t[:].unsqueeze(1).broadcast_to([128, nj, BH, W])

            def store(i: int, j: int, mt, r0: int = 0, r1: int = BH, eng=None) -> None:
                # eng: the engine whose queue dispatches the store DMA.
                # Stores must be issued on a queue in data-ready order —
                # a DMA waiting on its data semaphore head-of-line blocks
                # every later DMA on the same queue — so the DVE-leaky
                # path stores go on the DVE queue, ACT-path on SP.
                eng = eng or nc.sync
                dj = j - MD
                if dj == 0:
                    eng.dma_start(
                        out=out_d[:, i, j, r0:r1], in_=mt[:, j, r0:r1]
                    )
                elif dj < 0:
                    # rows y < -dj are zero in the yb=0 block (partitions 0:32)
                    eng.dma_start(
                        out=out_d[IPC:, i, j, r0:r1], in_=mt[IPC:, j, r0:r1]
                    )
                    z0 = max(r0, -dj)
                    if z0 < r1:
                        eng.dma_start(
                            out=out_d[:IPC, i, j, z0:r1], in_=mt[:IPC, j, z0:r1]
                        )
                else:
                    # rows y >= BH-dj are zero in the yb=3 block (parts 96:128)
                    eng.dma_start(
                        out=out_d[: 3 * IPC, i, j, r0:r1], in_=mt[: 3 * IPC, j, r0:r1]
                    )
                    z1 = min(r1, BH - dj)
                    if r0 < z1:
                        eng.dma_start(
                            out=out_d[3 * IPC :, i, j, r0:z1],
                            in_=mt[3 * IPC :, j, r0:z1],
                        )

            def mult(i, j0, j1, mt, r0=0, r1=BH):
                # 1-j chunks use a plain slice AP: the manual window AP
                # defeats subtile dependency tracking (conservative
                # whole-tile dep on tgt_t), which matters while the input
                # loads are still in flight.
                if j1 - j0 == 1:
                    src = tgt_t[:, j0 + r0 : j0 + r1, i : i + W].unsqueeze(1)
                    ob = mt[:, j0:j1, r0:r1]
                    rb = ref_t[:, r0:r1].unsqueeze(1)
                else:
                    assert (r0, r1) == (0, BH)
                    src = win(i, j0, j1)
                    ob = mt[:, j0:j1]
                    rb = ref_t[:].unsqueeze(1)
                nc.vector.tensor_tensor(
                    out=ob,
                    in0=rb.broadcast_to([128, j1 - j0, r1 - r0, W]),
                    in1=src,
                    op=mybir.AluOpType.mult,
                )

            def xr(i):
                # Structurally-zero x columns for width offset i: the mult
                # writes exact zeros there (zero-padded halo), so leaky can
                # skip them — leaky(0) == 0 and stores ship them unchanged.
                di = i - MD
                return max(0, -di), W - max(0, di)

            def leaky_act(i, j0, j1, mt, r0=0, r1=BH):
                x0, x1 = xr(i)
                nc.scalar.activation(
                    out=mt[:, j0:j1, r0:r1, x0:x1],
                    in_=mt[:, j0:j1, r0:r1, x0:x1],
                    func=mybir.ActivationFunctionType.Prelu,
                    alpha=0.1,
                )
                for j in range(j0, j1):
                    store(i, j, mt, r0, r1)

            def dve_ts(i, j0, j1, mt, amt):
                # First half of the DVE leaky: am = 0.1*m (4x perf mode).
                # (The Pool/GPSIMD engine rejects ALU tensor ops at the ISA
                # check, so the only elementwise engines are ACT and DVE.)
                nj = j1 - j0
                x0, x1 = xr(i)
                nc.vector.tensor_scalar(
                    out=amt[:, :nj, :, : x1 - x0],
                    in0=mt[:, j0:j1, :, x0:x1],
                    scalar1=0.1,
                    scalar2=None,
                    op0=mybir.AluOpType.mult,
                )

            def dve_max(i, j0, j1, mt, amt):
                # Second half: leaky = max(m, am) (2x mode), then stores on
                # the idle Pool/SWDGE queue so they cannot head-of-line
                # block the SP store queue.
                nj = j1 - j0
                x0, x1 = xr(i)
                nc.vector.tensor_tensor(
                    out=mt[:, j0:j1, :, x0:x1],
                    in0=mt[:, j0:j1, :, x0:x1],
                    in1=amt[:, :nj, :, : x1 - x0],
                    op=mybir.AluOpType.max,
                )
                for j in range(j0, j1):
                    store(i, j, mt, eng=nc.gpsimd)

            def leaky_dve(i, j0, j1, mt, amt):
                dve_ts(i, j0, j1, mt, amt)
                dve_max(i, j0, j1, mt, amt)

            def dve_ts2(i, j0, j1, mt, amt):
                nj = j1 - j0
                x0, x1 = xr(i)
                nc.vector.tensor_scalar(
                    out=amt[:, 2 : 2 + nj, :, : x1 - x0],
                    in0=mt[:, j0:j1, :, x0:x1],
                    scalar1=0.1,
                    scalar2=None,
                    op0=mybir.AluOpType.mult,
                )

            def dve_max2(i, j0, j1, mt, amt):
                nj = j1 - j0
                x0, x1 = xr(i)
                nc.vector.tensor_tensor(
                    out=mt[:, j0:j1, :, x0:x1],
                    in0=mt[:, j0:j1, :, x0:x1],
                    in1=amt[:, 2 : 2 + nj, :, : x1 - x0],
                    op=mybir.AluOpType.max,
                )
                for j in range(j0, j1):
                    store(i, j, mt, eng=nc.gpsimd)

            dummy_t = cpool.tile([128, 4], BF16)

            def dve_dummy():
                # Dependency-free filler op issued between the ts and max
                # of a DVE leaky pair: the list scheduler would otherwise
                # insert the next column's multi-us multiply there (max is
                # not "ready" until ts's semaphore propagates, ~100ns), and
                # the dummy gives that propagation time at ~160ns cost.
                nc.vector.memset(dummy_t[:], 0.0)

            # Leaky split: ACT takes ~7 j's per column, DVE (tensor_scalar
            # + max) the rest, tuned so DVE/ACT/DMA all land near 117us.
            # Chunk boundaries: fine chunks early (short latency -> first
            # stores fire ~8us in), larger chunks in steady state.
            for i in range(D):
                mt = mpool.tile([128, D, BH, W], BF16)
                amt = apool.tile([128, 3, BH, W], BF16)
                for step in PLAN[i]:
                    kind = step[0]
                    if kind == "d":
                        dve_dummy()
                        continue
                    j0, j1 = step[1], step[2]
                    r0, r1 = (step[3], step[4]) if len(step) > 3 else (0, BH)
                    if kind == "m":
                        mult(i, j0, j1, mt, r0, r1)
                    elif kind == "A":
                        leaky_act(i, j0, j1, mt, r0, r1)
                    elif kind == "t":
                        dve_ts(i, j0, j1, mt, amt)
                    elif kind == "x":
                        dve_max(i, j0, j1, mt, amt)
                    elif kind == "t2":
                        dve_ts2(i, j0, j1, mt, amt)
                    elif kind == "x2":
                        dve_max2(i, j0, j1, mt, amt)
                    else:
                        leaky_dve(i, j0, j1, mt, amt)
    nc.finalize()
    return nc


_cached_nc = None
_last_results = None


def _prep_inputs(ref: np.ndarray, tgt: np.ndarray):
    """ref/tgt: (256, 64, 128) f32 -> per-core blocked/halo'd bf16 arrays.

    Returns ref_blocked (8, 128, 16, 128) and tgt_halo (8, 128, 24, 136),
    partition p = yb*32 + n.
    """
    ref_b = ref.reshape(N_CORES, IPC, YB, BH, W).transpose(0, 2, 1, 3, 4)
    ref_b = np.ascontiguousarray(ref_b).reshape(N_CORES, 128, BH, W)

    tp = np.zeros((IMGS, H + 2 * MD, HALO_W), dtype=np.float32)
    tp[:, MD : MD + H, MD : MD + W] = tgt
    # overlapping 24-row windows starting at yb*16
    idx = (BH * np.arange(YB))[:, None] + np.arange(HALO_H)[None, :]
    halo = tp[:, idx, :]  # (256, 4, 24, 136)
    halo = halo.reshape(N_CORES, IPC, YB, HALO_H, HALO_W).transpose(0, 2, 1, 3, 4)
    halo = np.ascontiguousarray(halo).reshape(N_CORES, 128, HALO_H, HALO_W)
    return ref_b.astype(NP_BF16), halo.astype(NP_BF16)


def kernel(refimg_fea: np.ndarray, targetimg_fea: np.ndarray) -> np.ndarray:
    global _cached_nc, _last_results
    ref = np.asarray(refimg_fea, dtype=np.float32).reshape(IMGS, H, W)
    tgt = np.asarray(targetimg_fea, dtype=np.float32).reshape(IMGS, H, W)
    ref_b, tgt_h = _prep_inputs(ref, tgt)
    if _cached_nc is None:
        _cached_nc = _build()
    nc = _cached_nc
    in_maps = [{"ref": ref_b[k], "tgt": tgt_h[k]} for k in range(N_CORES)]
    res = bass_utils.run_bass_kernel_spmd(nc, in_maps, core_ids=list(range(N_CORES)))
    _last_results = res
    # Per-core output is [yb*32+n, i, j, y_lo, x]; reassemble to
    # [n, i, j, (yb y_lo), x] per core, then stack cores along n.
    parts = []
    for r in res.results:
        o = np.asarray(r["out"]).astype(np.float32)
        o = o.reshape(YB, IPC, D, D, BH, W)
        parts.append(o.transpose(1, 2, 3, 0, 4, 5).reshape(IPC, D, D, H, W))
    out = np.concatenate(parts, axis=0)
    return out.reshape(B, C, D, D, H, W)


# revision 30
# speedup vs baseline: 1.8503x; 1.0156x over previous
"""Correlation (9x9 displacement) kernel for Trainium2.

out[b,c,i,j,y,x] = leaky_relu(ref[b,c,y,x] * tgt[b,c, y+j-4, x+i-4], 0.1)
with zero padding outside the target image bounds.

Sharding: the 256 (b,c) images are split 32-per-core across 8 NeuronCores
(pure data parallel, no collectives).

Per-core layout: partition p = yb*32 + n  (yb = row-block 0..3, n = image
0..31).  Each partition stores a halo tile of the target: 24 rows x 136 cols
(its 16-row block plus +-4 halo rows, W plus +-4 pad cols, zeros outside the
image).  Every displacement (i,j) then becomes the plain in-bounds slice
tgt[:, j:j+16, i:i+128], and out-of-bounds zeros compute themselves via
leaky(tgt_pad*0) == 0.

The whole pipeline runs in bf16 (the correctness gate is rel_err < 2e-2;
the bf16 rounding chain stays near ~1e-2 worst case / ~4e-3 typical):
 - halves the output-store DMA bytes, which is the hard roofline here
   (the cost model serializes all DMA transfers at 360 B/ns per core), and
 - enables the DVE 2x/4x perf modes for the elementwise work.

Per i-column (fixed width offset i), one DVE tensor_tensor computes all 9
j-shifts at once via a 4D overlapping-window AP on the halo tile (the j and
y dims share the row stride) with a stride-0 broadcast AP for ref.  The
leaky is split across engines to stay under the DMA roofline:
  j=0..5 -> ACT Prelu (in-place),
  j=6..8 -> DVE tensor_scalar (0.1*m) + Pool tensor_tensor max(m, 0.1m),
using leaky(v) == max(v, 0.1*v) for slope 0.1.
Column i=0 is issued fine-grained (1/2/3-j chunks) so the first store fires
~8us in instead of ~25us (pipeline fill).

Structural zero rows (y+j-4 out of [0,64)) are skipped on the store; the
DRAM output buffer is pre-zeroed, and the zeros live in one 32-partition
block (yb=0 for j<4, yb=3 for j>4), so each such store splits in two.
"""

import ml_dtypes
import numpy as np

import concourse.bacc as bacc
import concourse.mybir as mybir
from concourse import bass_utils
from concourse.ap import AP
from concourse.tile import TileContext

B, C, H, W = 4, 64, 64, 128
MD = 4
D = 2 * MD + 1  # 9
N_CORES = 8
IMGS = B * C  # 256
IPC = IMGS // N_CORES  # 32 images per core
YB = 4  # row blocks per image
BH = H // YB  # 16 rows per block
HALO_H = BH + 2 * MD  # 24
HALO_W = W + 2 * MD  # 136
BF16 = mybir.dt.bfloat16
NP_BF16 = ml_dtypes.bfloat16

# Packed output widths: for width offset i, |i-MD| x-columns are structural
# zeros (out of bounds); neither computed, leaky'd, nor stored.  Column i's
# tiles and DRAM segment are WI[i] wide; the host re-inserts the zeros.
WI = [W - abs(i - MD) for i in range(D)]  # 124..128..124
X0 = [max(0, MD - i) for i in range(D)]  # first valid x per column
_SEG = [D * BH * w for w in WI]
BASE = [sum(_SEG[:i]) for i in range(D)]
FLAT = sum(_SEG)  # 163008 elems per partition


# Per-column op plan for columns 1..8 (column 0 is hand-scheduled fine-
# grained warmup).  Each entry: ("m", j0, j1) DVE multiply chunk,
# ("A", j0, j1) ACT Prelu + stores, ("V", j0, j1) DVE scale+max + stores.
_STD = [
    ("m", 0, 4), ("A", 0, 2), ("A", 2, 4), ("m", 7, 9), ("t", 7, 9), ("d",),
    ("x", 7, 9), ("m", 4, 7), ("A", 4, 6), ("A", 6, 7),
]
_K1 = [
    ("m", 0, 4), ("A", 0, 2), ("A", 2, 4), ("m", 8, 9), ("t", 8, 9), ("d",),
    ("x", 8, 9), ("m", 4, 8), ("A", 4, 6), ("A", 6, 8),
]
_C1 = [
    ("m", 0, 2), ("A", 0, 2), ("m", 2, 4), ("A", 2, 4),
    ("m", 7, 9), ("t", 7, 9), ("d",), ("x", 7, 9),
    ("m", 4, 7), ("A", 4, 6), ("A", 6, 7),
]
_C0 = [
    ("m", 0, 1, 0, 8), ("A", 0, 1, 0, 8), ("m", 0, 1, 8, 16), ("A", 0, 1, 8, 16),
    ("m", 1, 2), ("A", 1, 2), ("m", 7, 8), ("V", 7, 8),
    ("m", 2, 3), ("A", 2, 3), ("m", 8, 9), ("t2", 8, 9), ("x2", 8, 9),
    ("m", 3, 4), ("A", 3, 4), ("m", 4, 5), ("A", 4, 5), ("m", 5, 7), ("A", 5, 7),
]
PLAN = {0: _C0, 1: _C1, 2: _STD, 3: _STD, 4: _K1, 5: _STD, 6: _STD, 7: _STD, 8: _STD}


def _build() -> "bacc.Bacc":
    nc = bacc.Bacc(trn_type="TRN2")
    ref_d = nc.dram_tensor("ref", [128, BH, W], BF16, kind="ExternalInput")
    tgt_d = nc.dram_tensor("tgt", [128, HALO_H, HALO_W], BF16, kind="ExternalInput")
    # Partition-major packed output: per partition p = yb*32 + n, column i's
    # segment lives at BASE[i] with row width WI[i]; the host untangles
    # (yb, n) and re-inserts the zero x-borders during unsharding.
    out_d = nc.dram_tensor("out", [128, FLAT], BF16, kind="ExternalOutput")

    with TileContext(nc) as tc:
        with (
            tc.tile_pool(name="const", bufs=1) as cpool,
            tc.tile_pool(name="m", bufs=3) as mpool,
            tc.tile_pool(name="am", bufs=3) as apool,
        ):
            tgt_t = cpool.tile([128, HALO_H, HALO_W], BF16)
            ref_t = cpool.tile([128, BH, W], BF16)
            # Fine-grained loads so the first compute chunk (j=0, rows 0:8)
            # only waits for the first two transfers.  (No finer: HWDGE
            # costs 625ns per DMA, so below ~500B/partition the input
            # loading becomes descriptor-generation-bound.)
            HB = BH // 2  # 8
            nc.sync.dma_start(out=tgt_t[:, :HB], in_=tgt_d[:, :HB])
            nc.sync.dma_start(out=ref_t[:, :HB], in_=ref_d[:, :HB])
            nc.sync.dma_start(out=ref_t[:, HB:], in_=ref_d[:, HB:])
            nc.sync.dma_start(out=tgt_t[:, HB:BH], in_=tgt_d[:, HB:BH])
            nc.sync.dma_start(out=tgt_t[:, BH:], in_=tgt_d[:, BH:])

            ta = tgt_t[:]

            def win(i: int, j0: int, j1: int) -> AP:
                # [p, j, y, x] overlapping-window view of the halo tile:
                # j and y share the row stride.  x starts at the first
                # valid (in-bounds) output column and spans WI[i].
                return AP(
                    tensor=ta.tensor,
                    offset=ta.offset + j0 * HALO_W + i + X0[i],
                    ap=[list(ta.ap[0]), [HALO_W, j1 - j0], [HALO_W, BH], [1, WI[i]]],
                )

            def store(i: int, j: int, mt, r0: int = 0, r1: int = BH, eng=None) -> None:
                # eng: the engine whose queue dispatches the store DMA.
                # Stores must be issued on a queue in data-ready order —
                # a DMA waiting on its data semaphore head-of-line blocks
                # every later DMA on the same queue — so the DVE-leaky
                # path stores go on the idle Pool queue, ACT-path on SP.
                eng = eng or nc.sync
                w = WI[i]
                o0 = BASE[i] + j * BH * w
                dj = j - MD

                def seg(p0, p1, a, b):
                    if a < b:
                        eng.dma_start(
                            out=out_d[p0:p1, o0 + a * w : o0 + b * w],
                            in_=mt[p0:p1, j, a:b],
                        )

                if dj == 0:
                    seg(0, 128, r0, r1)
                elif dj < 0:
                    # rows y < -dj are zero in the yb=0 block (partitions 0:32)
                    seg(IPC, 128, r0, r1)
                    seg(0, IPC, max(r0, -dj), r1)
                else:
                    # rows y >= BH-dj are zero in the yb=3 block (parts 96:128)
                    seg(0, 3 * IPC, r0, r1)
                    seg(3 * IPC, 128, r0, min(r1, BH - dj))

            def mult(i, j0, j1, mt, r0=0, r1=BH):
                # 1-j chunks use a plain slice AP: the manual window AP
                # defeats subtile dependency tracking (conservative
                # whole-tile dep on tgt_t), which matters while the input
                # loads are still in flight.
                w = WI[i]
                if j1 - j0 == 1:
                    c0 = i + X0[i]
                    src = tgt_t[:, j0 + r0 : j0 + r1, c0 : c0 + w].unsqueeze(1)
                    ob = mt[:, j0:j1, r0:r1]
                    rb = ref_t[:, r0:r1, X0[i] : X0[i] + w].unsqueeze(1)
                else:
                    assert (r0, r1) == (0, BH)
                    src = win(i, j0, j1)
                    ob = mt[:, j0:j1]
                    rb = ref_t[:, :, X0[i] : X0[i] + w].unsqueeze(1)
                nc.vector.tensor_tensor(
                    out=ob,
                    in0=rb.broadcast_to([128, j1 - j0, r1 - r0, w]),
                    in1=src,
                    op=mybir.AluOpType.mult,
                )

            def leaky_act(i, j0, j1, mt, r0=0, r1=BH):
                nc.scalar.activation(
                    out=mt[:, j0:j1, r0:r1],
                    in_=mt[:, j0:j1, r0:r1],
                    func=mybir.ActivationFunctionType.Prelu,
                    alpha=0.1,
                )
                for j in range(j0, j1):
                    store(i, j, mt, r0, r1)

            def dve_ts(i, j0, j1, mt, amt, slot=0):
                # First half of the DVE leaky: am = 0.1*m (4x perf mode).
                # (The Pool/GPSIMD engine rejects ALU tensor ops at the ISA
                # check, so the only elementwise engines are ACT and DVE.)
                nj = j1 - j0
                nc.vector.tensor_scalar(
                    out=amt[:, slot : slot + nj, :, : WI[i]],
                    in0=mt[:, j0:j1],
                    scalar1=0.1,
                    scalar2=None,
                    op0=mybir.AluOpType.mult,
                )

            def dve_max(i, j0, j1, mt, amt, slot=0):
                # Second half: leaky = max(m, am) (2x mode), then stores on
                # the idle Pool/SWDGE queue so they cannot head-of-line
                # block the SP store queue.
                nj = j1 - j0
                nc.vector.tensor_tensor(
                    out=mt[:, j0:j1],
                    in0=mt[:, j0:j1],
                    in1=amt[:, slot : slot + nj, :, : WI[i]],
                    op=mybir.AluOpType.max,
                )
                for j in range(j0, j1):
                    store(i, j, mt, eng=nc.gpsimd)

            def leaky_dve(i, j0, j1, mt, amt):
                dve_ts(i, j0, j1, mt, amt)
                dve_max(i, j0, j1, mt, amt)

            def dve_ts2(i, j0, j1, mt, amt):
                dve_ts(i, j0, j1, mt, amt, slot=2)

            def dve_max2(i, j0, j1, mt, amt):
                dve_max(i, j0, j1, mt, amt, slot=2)

            dummy_t = cpool.tile([128, 4], BF16)

            def dve_dummy():
                # Dependency-free filler op issued between the ts and max
                # of a DVE leaky pair: the list scheduler would otherwise
                # insert the next column's multi-us multiply there (max is
                # not "ready" until ts's semaphore propagates, ~100ns), and
                # the dummy gives that propagation time at ~160ns cost.
                nc.vector.memset(dummy_t[:], 0.0)

            # Leaky split: ACT takes ~7 j's per column, DVE (tensor_scalar
            # + max) the rest, tuned so DVE/ACT/DMA all land near 117us.
            # Chunk boundaries: fine chunks early (short latency -> first
            # stores fire ~8us in), larger chunks in steady state.
            for i in range(D):
                mt = mpool.tile([128, D, BH, WI[i]], BF16)
                amt = apool.tile([128, 3, BH, W], BF16)
                for step in PLAN[i]:
                    kind = step[0]
                    if kind == "d":
                        dve_dummy()
                        continue
                    j0, j1 = step[1], step[2]
                    r0, r1 = (step[3], step[4]) if len(step) > 3 else (0, BH)
                    if kind == "m":
                        mult(i, j0, j1, mt, r0, r1)
                    elif kind == "A":
                        leaky_act(i, j0, j1, mt, r0, r1)
                    elif kind == "t":
                        dve_ts(i, j0, j1, mt, amt)
                    elif kind == "x":
                        dve_max(i, j0, j1, mt, amt)
                    elif kind == "t2":
                        dve_ts2(i, j0, j1, mt, amt)
                    elif kind == "x2":
                        dve_max2(i, j0, j1, mt, amt)
                    else:
                        leaky_dve(i, j0, j1, mt, amt)
    nc.finalize()
    return nc


_cached_nc = None
_last_results = None


def _prep_inputs(ref: np.ndarray, tgt: np.ndarray):
    """ref/tgt: (256, 64, 128) f32 -> per-core blocked/halo'd bf16 arrays.

    Returns ref_blocked (8, 128, 16, 128) and tgt_halo (8, 128, 24, 136),
    partition p = yb*32 + n.
    """
    ref_b = ref.reshape(N_CORES, IPC, YB, BH, W).transpose(0, 2, 1, 3, 4)
    ref_b = np.ascontiguousarray(ref_b).reshape(N_CORES, 128, BH, W)

    tp = np.zeros((IMGS, H + 2 * MD, HALO_W), dtype=np.float32)
    tp[:, MD : MD + H, MD : MD + W] = tgt
    # overlapping 24-row windows starting at yb*16
    idx = (BH * np.arange(YB))[:, None] + np.arange(HALO_H)[None, :]
    halo = tp[:, idx, :]  # (256, 4, 24, 136)
    halo = halo.reshape(N_CORES, IPC, YB, HALO_H, HALO_W).transpose(0, 2, 1, 3, 4)
    halo = np.ascontiguousarray(halo).reshape(N_CORES, 128, HALO_H, HALO_W)
    return ref_b.astype(NP_BF16), halo.astype(NP_BF16)


def kernel(refimg_fea: np.ndarray, targetimg_fea: np.ndarray) -> np.ndarray:
    global _cached_nc, _last_results
    ref = np.asarray(refimg_fea, dtype=np.float32).reshape(IMGS, H, W)
    tgt = np.asarray(targetimg_fea, dtype=np.float32).reshape(IMGS, H, W)
    ref_b, tgt_h = _prep_inputs(ref, tgt)
    if _cached_nc is None:
        _cached_nc = _build()
    nc = _cached_nc
    in_maps = [{"ref": ref_b[k], "tgt": tgt_h[k]} for k in range(N_CORES)]
    res = bass_utils.run_bass_kernel_spmd(nc, in_maps, core_ids=list(range(N_CORES)))
    _last_results = res
    # Per-core output is [yb*32+n, i, j, y_lo, x]; reassemble to
    # [n, i, j, (yb y_lo), x] per core, then stack cores along n.
    parts = []
    for r in res.results:
        flat = np.asarray(r["out"]).astype(np.float32)  # (128, FLAT) packed
        o = np.zeros((128, D, D, BH, W), dtype=np.float32)
        for i in range(D):
            w = WI[i]
            seg = flat[:, BASE[i] : BASE[i] + D * BH * w].reshape(128, D, BH, w)
            o[:, i, :, :, X0[i] : X0[i] + w] = seg
        o = o.reshape(YB, IPC, D, D, BH, W)
        parts.append(o.transpose(1, 2, 3, 0, 4, 5).reshape(IPC, D, D, H, W))
    out = np.concatenate(parts, axis=0)
    return out.reshape(B, C, D, D, H, W)


# revision 33
# speedup vs baseline: 1.8533x; 1.0016x over previous
"""Correlation (9x9 displacement) kernel for Trainium2.

out[b,c,i,j,y,x] = leaky_relu(ref[b,c,y,x] * tgt[b,c, y+j-4, x+i-4], 0.1)
with zero padding outside the target image bounds.

Sharding: the 256 (b,c) images are split 32-per-core across 8 NeuronCores
(pure data parallel, no collectives).

Per-core layout: partition p = yb*32 + n  (yb = row-block 0..3, n = image
0..31).  Each partition stores a halo tile of the target: 24 rows x 136 cols
(its 16-row block plus +-4 halo rows, W plus +-4 pad cols, zeros outside the
image).  Every displacement (i,j) then becomes the plain in-bounds slice
tgt[:, j:j+16, i:i+128], and out-of-bounds zeros compute themselves via
leaky(tgt_pad*0) == 0.

The whole pipeline runs in bf16 (the correctness gate is rel_err < 2e-2;
the bf16 rounding chain measures 1.3e-2 max on these inputs):
 - halves the output-store DMA bytes, which is the hard roofline here
   (the cost model serializes all DMA transfers at 360 B/ns per core), and
 - enables the DVE 2x/4x perf modes for the elementwise work.

Per i-column (fixed width offset i), DVE tensor_tensor computes batches of
j-shifts via a 4D overlapping-window AP on the halo tile (the j and y dims
share the row stride) with a stride-0 broadcast AP for ref.  The leaky
(leaky(v) == max(v, 0.1*v) for slope 0.1) is split across engines so
DVE/ACT/DMA all land near 115.7us:
  7 j's per column -> ACT Prelu (in-place),
  2 j's per column -> DVE tensor_scalar 0.1*m (4x mode) + tensor_tensor
  max (2x mode); their stores dispatch from the idle Pool/SWDGE queue so
  they cannot head-of-line block the SP store queue.
(The Pool/GPSIMD engine rejects ALU tensor ops at the neuronxcc ISA check,
so ACT and DVE are the only elementwise engines.)

Structural zeros are never computed, leaky'd, or stored:
 - zero rows (y+j-4 out of [0,64)) are skipped on the store — they live in
   one 32-partition block (yb=0 for j<4, yb=3 for j>4), so each such store
   splits in two; the DRAM output buffer is pre-zeroed;
 - zero x-columns (x+i-4 out of [0,128)) are packed out entirely: column
   i's tiles and DRAM segment are WI[i] in (124..128) wide.

Column 0 is issued fine-grained (quarter/half-row first chunks, per-j
after) so the first store fires ~7us in instead of ~25us (pipeline fill),
and every column leads with 2-j chunks to keep store release smooth —
production and the 360 B/ns DMA drain are phase-matched within ~1.5us
after warmup.
"""

import ml_dtypes
import numpy as np

import concourse.bacc as bacc
import concourse.mybir as mybir
from concourse import bass_utils
from concourse.ap import AP
from concourse.tile import TileContext

B, C, H, W = 4, 64, 64, 128
MD = 4
D = 2 * MD + 1  # 9
N_CORES = 8
IMGS = B * C  # 256
IPC = IMGS // N_CORES  # 32 images per core
YB = 4  # row blocks per image
BH = H // YB  # 16 rows per block
HALO_H = BH + 2 * MD  # 24
HALO_W = W + 2 * MD  # 136
BF16 = mybir.dt.bfloat16
NP_BF16 = ml_dtypes.bfloat16

# Packed output widths: for width offset i, |i-MD| x-columns are structural
# zeros (out of bounds); neither computed, leaky'd, nor stored.  Column i's
# tiles and DRAM segment are WI[i] wide; the host re-inserts the zeros.
WI = [W - abs(i - MD) for i in range(D)]  # 124..128..124
X0 = [max(0, MD - i) for i in range(D)]  # first valid x per column
_SEG = [D * BH * w for w in WI]
BASE = [sum(_SEG[:i]) for i in range(D)]
FLAT = sum(_SEG)  # 163008 elems per partition


# Per-column op plan for columns 1..8 (column 0 is hand-scheduled fine-
# grained warmup).  Each entry: ("m", j0, j1) DVE multiply chunk,
# ("A", j0, j1) ACT Prelu + stores, ("V", j0, j1) DVE scale+max + stores.
_STD = [
    ("m", 0, 2), ("A", 0, 2), ("m", 2, 4), ("A", 2, 4),
    ("m", 7, 9), ("t", 7, 9), ("d",), ("x", 7, 9),
    ("m", 4, 7), ("A", 4, 6), ("A", 6, 7),
]
_K1 = [
    ("m", 0, 4), ("A", 0, 2), ("A", 2, 4), ("m", 8, 9), ("t", 8, 9), ("d",),
    ("x", 8, 9), ("m", 4, 8), ("A", 4, 6), ("A", 6, 8),
]
_C0 = [
    ("m", 0, 1, 0, 4), ("A", 0, 1, 0, 4), ("m", 0, 1, 4, 8), ("A", 0, 1, 4, 8),
    ("m", 0, 1, 8, 16), ("A", 0, 1, 8, 16),
    ("m", 1, 2), ("A", 1, 2), ("m", 7, 8), ("V", 7, 8),
    ("m", 2, 3), ("A", 2, 3), ("m", 8, 9), ("t2", 8, 9), ("x2", 8, 9),
    ("m", 3, 4), ("A", 3, 4), ("m", 4, 5), ("A", 4, 5), ("m", 5, 7), ("A", 5, 7),
]
PLAN = {0: _C0, 1: _STD, 2: _STD, 3: _STD, 4: _K1, 5: _STD, 6: _STD, 7: _STD, 8: _STD}


def _build() -> "bacc.Bacc":
    nc = bacc.Bacc(trn_type="TRN2")
    ref_d = nc.dram_tensor("ref", [128, BH, W], BF16, kind="ExternalInput")
    tgt_d = nc.dram_tensor("tgt", [128, HALO_H, HALO_W], BF16, kind="ExternalInput")
    # Partition-major packed output: per partition p = yb*32 + n, column i's
    # segment lives at BASE[i] with row width WI[i]; the host untangles
    # (yb, n) and re-inserts the zero x-borders during unsharding.
    out_d = nc.dram_tensor("out", [128, FLAT], BF16, kind="ExternalOutput")

    with TileContext(nc) as tc:
        with (
            tc.tile_pool(name="const", bufs=1) as cpool,
            tc.tile_pool(name="m", bufs=3) as mpool,
            tc.tile_pool(name="am", bufs=3) as apool,
        ):
            tgt_t = cpool.tile([128, HALO_H, HALO_W], BF16)
            ref_t = cpool.tile([128, BH, W], BF16)
            # Fine-grained loads so the first compute chunk (j=0, rows 0:8)
            # only waits for the first two transfers.  (No finer: HWDGE
            # costs 625ns per DMA, so below ~500B/partition the input
            # loading becomes descriptor-generation-bound.)
            HB = BH // 2  # 8
            nc.sync.dma_start(out=tgt_t[:, :HB], in_=tgt_d[:, :HB])
            nc.sync.dma_start(out=ref_t[:, :HB], in_=ref_d[:, :HB])
            nc.sync.dma_start(out=ref_t[:, HB:], in_=ref_d[:, HB:])
            nc.sync.dma_start(out=tgt_t[:, HB:BH], in_=tgt_d[:, HB:BH])
            nc.sync.dma_start(out=tgt_t[:, BH:], in_=tgt_d[:, BH:])

            ta = tgt_t[:]

            def win(i: int, j0: int, j1: int) -> AP:
                # [p, j, y, x] overlapping-window view of the halo tile:
                # j and y share the row stride.  x starts at the first
                # valid (in-bounds) output column and spans WI[i].
                return AP(
                    tensor=ta.tensor,
                    offset=ta.offset + j0 * HALO_W + i + X0[i],
                    ap=[list(ta.ap[0]), [HALO_W, j1 - j0], [HALO_W, BH], [1, WI[i]]],
                )

            def store(i: int, j: int, mt, r0: int = 0, r1: int = BH, eng=None) -> None:
                # eng: the engine whose queue dispatches the store DMA.
                # Stores must be issued on a queue in data-ready order —
                # a DMA waiting on its data semaphore head-of-line blocks
                # every later DMA on the same queue — so the DVE-leaky
                # path stores go on the idle Pool queue, ACT-path on SP.
                eng = eng or nc.sync
                w = WI[i]
                o0 = BASE[i] + j * BH * w
                dj = j - MD

                def seg(p0, p1, a, b):
                    if a < b:
                        eng.dma_start(
                            out=out_d[p0:p1, o0 + a * w : o0 + b * w],
                            in_=mt[p0:p1, j, a:b],
                        )

                if dj == 0:
                    seg(0, 128, r0, r1)
                elif dj < 0:
                    # rows y < -dj are zero in the yb=0 block (partitions 0:32)
                    seg(IPC, 128, r0, r1)
                    seg(0, IPC, max(r0, -dj), r1)
                else:
                    # rows y >= BH-dj are zero in the yb=3 block (parts 96:128)
                    seg(0, 3 * IPC, r0, r1)
                    seg(3 * IPC, 128, r0, min(r1, BH - dj))

            def mult(i, j0, j1, mt, r0=0, r1=BH):
                # 1-j chunks use a plain slice AP: the manual window AP
                # defeats subtile dependency tracking (conservative
                # whole-tile dep on tgt_t), which matters while the input
                # loads are still in flight.
                w = WI[i]
                if j1 - j0 == 1:
                    c0 = i + X0[i]
                    src = tgt_t[:, j0 + r0 : j0 + r1, c0 : c0 + w].unsqueeze(1)
                    ob = mt[:, j0:j1, r0:r1]
                    rb = ref_t[:, r0:r1, X0[i] : X0[i] + w].unsqueeze(1)
                else:
                    assert (r0, r1) == (0, BH)
                    src = win(i, j0, j1)
                    ob = mt[:, j0:j1]
                    rb = ref_t[:, :, X0[i] : X0[i] + w].unsqueeze(1)
                nc.vector.tensor_tensor(
                    out=ob,
                    in0=rb.broadcast_to([128, j1 - j0, r1 - r0, w]),
                    in1=src,
                    op=mybir.AluOpType.mult,
                )

            def leaky_act(i, j0, j1, mt, r0=0, r1=BH):
                nc.scalar.activation(
                    out=mt[:, j0:j1, r0:r1],
                    in_=mt[:, j0:j1, r0:r1],
                    func=mybir.ActivationFunctionType.Prelu,
                    alpha=0.1,
                )
                for j in range(j0, j1):
                    store(i, j, mt, r0, r1)

            def dve_ts(i, j0, j1, mt, amt, slot=0):
                # First half of the DVE leaky: am = 0.1*m (4x perf mode).
                # (The Pool/GPSIMD engine rejects ALU tensor ops at the ISA
                # check, so the only elementwise engines are ACT and DVE.)
                nj = j1 - j0
                nc.vector.tensor_scalar(
                    out=amt[:, slot : slot + nj, :, : WI[i]],
                    in0=mt[:, j0:j1],
                    scalar1=0.1,
                    scalar2=None,
                    op0=mybir.AluOpType.mult,
                )

            def dve_max(i, j0, j1, mt, amt, slot=0):
                # Second half: leaky = max(m, am) (2x mode), then stores on
                # the idle Pool/SWDGE queue so they cannot head-of-line
                # block the SP store queue.
                nj = j1 - j0
                nc.vector.tensor_tensor(
                    out=mt[:, j0:j1],
                    in0=mt[:, j0:j1],
                    in1=amt[:, slot : slot + nj, :, : WI[i]],
                    op=mybir.AluOpType.max,
                )
                for j in range(j0, j1):
                    store(i, j, mt, eng=nc.gpsimd)

            def leaky_dve(i, j0, j1, mt, amt):
                dve_ts(i, j0, j1, mt, amt)
                dve_max(i, j0, j1, mt, amt)

            def dve_ts2(i, j0, j1, mt, amt):
                dve_ts(i, j0, j1, mt, amt, slot=2)

            def dve_max2(i, j0, j1, mt, amt):
                dve_max(i, j0, j1, mt, amt, slot=2)

            dummy_t = cpool.tile([128, 4], BF16)

            def dve_dummy():
                # Dependency-free filler op issued between the ts and max
                # of a DVE leaky pair: the list scheduler would otherwise
                # insert the next column's multi-us multiply there (max is
                # not "ready" until ts's semaphore propagates, ~100ns), and
                # the dummy gives that propagation time at ~160ns cost.
                nc.vector.memset(dummy_t[:], 0.0)

            # Leaky split: ACT takes ~7 j's per column, DVE (tensor_scalar
            # + max) the rest, tuned so DVE/ACT/DMA all land near 117us.
            # Chunk boundaries: fine chunks early (short latency -> first
            # stores fire ~8us in), larger chunks in steady state.
            for i in range(D):
                mt = mpool.tile([128, D, BH, WI[i]], BF16)
                amt = apool.tile([128, 3, BH, W], BF16)
                for step in PLAN[i]:
                    kind = step[0]
                    if kind == "d":
                        dve_dummy()
                        continue
                    j0, j1 = step[1], step[2]
                    r0, r1 = (step[3], step[4]) if len(step) > 3 else (0, BH)
                    if kind == "m":
                        mult(i, j0, j1, mt, r0, r1)
                    elif kind == "A":
                        leaky_act(i, j0, j1, mt, r0, r1)
                    elif kind == "t":
                        dve_ts(i, j0, j1, mt, amt)
                    elif kind == "x":
                        dve_max(i, j0, j1, mt, amt)
                    elif kind == "t2":
                        dve_ts2(i, j0, j1, mt, amt)
                    elif kind == "x2":
                        dve_max2(i, j0, j1, mt, amt)
                    else:
                        leaky_dve(i, j0, j1, mt, amt)
    nc.finalize()
    return nc


_cached_nc = None
_last_results = None


def _prep_inputs(ref: np.ndarray, tgt: np.ndarray):
    """ref/tgt: (256, 64, 128) f32 -> per-core blocked/halo'd bf16 arrays.

    Returns ref_blocked (8, 128, 16, 128) and tgt_halo (8, 128, 24, 136),
    partition p = yb*32 + n.
    """
    ref_b = ref.reshape(N_CORES, IPC, YB, BH, W).transpose(0, 2, 1, 3, 4)
    ref_b = np.ascontiguousarray(ref_b).reshape(N_CORES, 128, BH, W)

    tp = np.zeros((IMGS, H + 2 * MD, HALO_W), dtype=np.float32)
    tp[:, MD : MD + H, MD : MD + W] = tgt
    # overlapping 24-row windows starting at yb*16
    idx = (BH * np.arange(YB))[:, None] + np.arange(HALO_H)[None, :]
    halo = tp[:, idx, :]  # (256, 4, 24, 136)
    halo = halo.reshape(N_CORES, IPC, YB, HALO_H, HALO_W).transpose(0, 2, 1, 3, 4)
    halo = np.ascontiguousarray(halo).reshape(N_CORES, 128, HALO_H, HALO_W)
    return ref_b.astype(NP_BF16), halo.astype(NP_BF16)


def kernel(refimg_fea: np.ndarray, targetimg_fea: np.ndarray) -> np.ndarray:
    global _cached_nc, _last_results
    ref = np.asarray(refimg_fea, dtype=np.float32).reshape(IMGS, H, W)
    tgt = np.asarray(targetimg_fea, dtype=np.float32).reshape(IMGS, H, W)
    ref_b, tgt_h = _prep_inputs(ref, tgt)
    if _cached_nc is None:
        _cached_nc = _build()
    nc = _cached_nc
    in_maps = [{"ref": ref_b[k], "tgt": tgt_h[k]} for k in range(N_CORES)]
    res = bass_utils.run_bass_kernel_spmd(nc, in_maps, core_ids=list(range(N_CORES)))
    _last_results = res
    # Per-core output is [yb*32+n, i, j, y_lo, x]; reassemble to
    # [n, i, j, (yb y_lo), x] per core, then stack cores along n.
    parts = []
    for r in res.results:
        flat = np.asarray(r["out"]).astype(np.float32)  # (128, FLAT) packed
        o = np.zeros((128, D, D, BH, W), dtype=np.float32)
        for i in range(D):
            w = WI[i]
            seg = flat[:, BASE[i] : BASE[i] + D * BH * w].reshape(128, D, BH, w)
            o[:, i, :, :, X0[i] : X0[i] + w] = seg
        o = o.reshape(YB, IPC, D, D, BH, W)
        parts.append(o.transpose(1, 2, 3, 0, 4, 5).reshape(IPC, D, D, H, W))
    out = np.concatenate(parts, axis=0)
    return out.reshape(B, C, D, D, H, W)


# revision 34
# speedup vs baseline: 1.8584x; 1.0028x over previous
"""Correlation (9x9 displacement) kernel for Trainium2.

out[b,c,i,j,y,x] = leaky_relu(ref[b,c,y,x] * tgt[b,c, y+j-4, x+i-4], 0.1)
with zero padding outside the target image bounds.

Sharding: the 256 (b,c) images are split 32-per-core across 8 NeuronCores
(pure data parallel, no collectives).

Per-core layout: partition p = yb*32 + n  (yb = row-block 0..3, n = image
0..31).  Each partition stores a halo tile of the target: 24 rows x 136 cols
(its 16-row block plus +-4 halo rows, W plus +-4 pad cols, zeros outside the
image).  Every displacement (i,j) then becomes the plain in-bounds slice
tgt[:, j:j+16, i:i+128], and out-of-bounds zeros compute themselves via
leaky(tgt_pad*0) == 0.

The whole pipeline runs in bf16 (the correctness gate is rel_err < 2e-2;
the bf16 rounding chain measures 1.3e-2 max on these inputs):
 - halves the output-store DMA bytes, which is the hard roofline here
   (the cost model serializes all DMA transfers at 360 B/ns per core), and
 - enables the DVE 2x/4x perf modes for the elementwise work.

Per i-column (fixed width offset i), DVE tensor_tensor computes batches of
j-shifts via a 4D overlapping-window AP on the halo tile (the j and y dims
share the row stride) with a stride-0 broadcast AP for ref.  The leaky
(leaky(v) == max(v, 0.1*v) for slope 0.1) is split across engines so
DVE/ACT/DMA all land near 115.7us:
  7 j's per column -> ACT Prelu (in-place),
  2 j's per column -> DVE tensor_scalar 0.1*m (4x mode) + tensor_tensor
  max (2x mode); their stores dispatch from the idle Pool/SWDGE queue so
  they cannot head-of-line block the SP store queue.
(The Pool/GPSIMD engine rejects ALU tensor ops at the neuronxcc ISA check,
so ACT and DVE are the only elementwise engines.)

Structural zeros are never computed, leaky'd, or stored:
 - zero rows (y+j-4 out of [0,64)) are skipped on the store — they live in
   one 32-partition block (yb=0 for j<4, yb=3 for j>4), so each such store
   splits in two; the DRAM output buffer is pre-zeroed;
 - zero x-columns (x+i-4 out of [0,128)) are packed out entirely: column
   i's tiles and DRAM segment are WI[i] in (124..128) wide.

Column 0 is issued fine-grained (quarter/half-row first chunks, per-j
after) so the first store fires ~7us in instead of ~25us (pipeline fill),
and every column leads with 2-j chunks to keep store release smooth —
production and the 360 B/ns DMA drain are phase-matched within ~1.5us
after warmup.
"""

import ml_dtypes
import numpy as np

import concourse.bacc as bacc
import concourse.mybir as mybir
from concourse import bass_utils
from concourse.ap import AP
from concourse.tile import TileContext

B, C, H, W = 4, 64, 64, 128
MD = 4
D = 2 * MD + 1  # 9
N_CORES = 8
IMGS = B * C  # 256
IPC = IMGS // N_CORES  # 32 images per core
YB = 4  # row blocks per image
BH = H // YB  # 16 rows per block
HALO_H = BH + 2 * MD  # 24
HALO_W = W + 2 * MD  # 136
BF16 = mybir.dt.bfloat16
NP_BF16 = ml_dtypes.bfloat16

# Packed output widths: for width offset i, |i-MD| x-columns are structural
# zeros (out of bounds); neither computed, leaky'd, nor stored.  Column i's
# tiles and DRAM segment are WI[i] wide; the host re-inserts the zeros.
WI = [W - abs(i - MD) for i in range(D)]  # 124..128..124
X0 = [max(0, MD - i) for i in range(D)]  # first valid x per column
_SEG = [D * BH * w for w in WI]
BASE = [sum(_SEG[:i]) for i in range(D)]
FLAT = sum(_SEG)  # 163008 elems per partition


# Per-column op plan for columns 1..8 (column 0 is hand-scheduled fine-
# grained warmup).  Each entry: ("m", j0, j1) DVE multiply chunk,
# ("A", j0, j1) ACT Prelu + stores, ("V", j0, j1) DVE scale+max + stores.
_STD = [
    ("m", 0, 2), ("A", 0, 2), ("m", 2, 4), ("A", 2, 4),
    ("m", 7, 9), ("t", 7, 9), ("d",), ("x", 7, 9),
    ("m", 4, 7), ("A", 4, 6), ("A", 6, 7),
]
_K1 = [
    ("m", 0, 4), ("A", 0, 2), ("A", 2, 4), ("m", 8, 9), ("t", 8, 9), ("d",),
    ("x", 8, 9), ("m", 4, 8), ("A", 4, 6), ("A", 6, 8),
]
_C0 = [
    ("m", 0, 1, 0, 4), ("A", 0, 1, 0, 4), ("m", 0, 1, 4, 8), ("A", 0, 1, 4, 8),
    ("m", 0, 1, 8, 16), ("A", 0, 1, 8, 16),
    ("m", 1, 2), ("A", 1, 2), ("m", 7, 8), ("V", 7, 8),
    ("m", 2, 3), ("A", 2, 3), ("m", 8, 9), ("t2", 8, 9), ("x2", 8, 9),
    ("m", 3, 4), ("A", 3, 4), ("m", 4, 5), ("A", 4, 5), ("m", 5, 7), ("A", 5, 7),
]
# The 8-ACT/1-DVE column sits early (col 3): during warmup ACT-side
# production is the scarcer resource, and the lighter DVE column there
# lets the multiply stream run ahead sooner.
PLAN = {0: _C0, 1: _STD, 2: _STD, 3: _K1, 4: _STD, 5: _STD, 6: _STD, 7: _STD, 8: _STD}


def _build() -> "bacc.Bacc":
    nc = bacc.Bacc(trn_type="TRN2")
    ref_d = nc.dram_tensor("ref", [128, BH, W], BF16, kind="ExternalInput")
    tgt_d = nc.dram_tensor("tgt", [128, HALO_H, HALO_W], BF16, kind="ExternalInput")
    # Partition-major packed output: per partition p = yb*32 + n, column i's
    # segment lives at BASE[i] with row width WI[i]; the host untangles
    # (yb, n) and re-inserts the zero x-borders during unsharding.
    out_d = nc.dram_tensor("out", [128, FLAT], BF16, kind="ExternalOutput")

    with TileContext(nc) as tc:
        with (
            tc.tile_pool(name="const", bufs=1) as cpool,
            tc.tile_pool(name="m", bufs=3) as mpool,
            tc.tile_pool(name="am", bufs=3) as apool,
        ):
            tgt_t = cpool.tile([128, HALO_H, HALO_W], BF16)
            ref_t = cpool.tile([128, BH, W], BF16)
            # Fine-grained loads so the first compute chunk (j=0, rows 0:8)
            # only waits for the first two transfers.  (No finer: HWDGE
            # costs 625ns per DMA, so below ~500B/partition the input
            # loading becomes descriptor-generation-bound.)
            HB = BH // 2  # 8
            nc.sync.dma_start(out=tgt_t[:, :HB], in_=tgt_d[:, :HB])
            nc.sync.dma_start(out=ref_t[:, :HB], in_=ref_d[:, :HB])
            nc.sync.dma_start(out=ref_t[:, HB:], in_=ref_d[:, HB:])
            nc.sync.dma_start(out=tgt_t[:, HB:BH], in_=tgt_d[:, HB:BH])
            nc.sync.dma_start(out=tgt_t[:, BH:], in_=tgt_d[:, BH:])

            ta = tgt_t[:]

            def win(i: int, j0: int, j1: int) -> AP:
                # [p, j, y, x] overlapping-window view of the halo tile:
                # j and y share the row stride.  x starts at the first
                # valid (in-bounds) output column and spans WI[i].
                return AP(
                    tensor=ta.tensor,
                    offset=ta.offset + j0 * HALO_W + i + X0[i],
                    ap=[list(ta.ap[0]), [HALO_W, j1 - j0], [HALO_W, BH], [1, WI[i]]],
                )

            def store(i: int, j: int, mt, r0: int = 0, r1: int = BH, eng=None) -> None:
                # eng: the engine whose queue dispatches the store DMA.
                # Stores must be issued on a queue in data-ready order —
                # a DMA waiting on its data semaphore head-of-line blocks
                # every later DMA on the same queue — so the DVE-leaky
                # path stores go on the idle Pool queue, ACT-path on SP.
                eng = eng or nc.sync
                w = WI[i]
                o0 = BASE[i] + j * BH * w
                dj = j - MD

                def seg(p0, p1, a, b):
                    if a < b:
                        eng.dma_start(
                            out=out_d[p0:p1, o0 + a * w : o0 + b * w],
                            in_=mt[p0:p1, j, a:b],
                        )

                if dj == 0:
                    seg(0, 128, r0, r1)
                elif dj < 0:
                    # rows y < -dj are zero in the yb=0 block (partitions 0:32)
                    seg(IPC, 128, r0, r1)
                    seg(0, IPC, max(r0, -dj), r1)
                else:
                    # rows y >= BH-dj are zero in the yb=3 block (parts 96:128)
                    seg(0, 3 * IPC, r0, r1)
                    seg(3 * IPC, 128, r0, min(r1, BH - dj))

            def mult(i, j0, j1, mt, r0=0, r1=BH):
                # 1-j chunks use a plain slice AP: the manual window AP
                # defeats subtile dependency tracking (conservative
                # whole-tile dep on tgt_t), which matters while the input
                # loads are still in flight.
                w = WI[i]
                if j1 - j0 == 1:
                    c0 = i + X0[i]
                    src = tgt_t[:, j0 + r0 : j0 + r1, c0 : c0 + w].unsqueeze(1)
                    ob = mt[:, j0:j1, r0:r1]
                    rb = ref_t[:, r0:r1, X0[i] : X0[i] + w].unsqueeze(1)
                else:
                    assert (r0, r1) == (0, BH)
                    src = win(i, j0, j1)
                    ob = mt[:, j0:j1]
                    rb = ref_t[:, :, X0[i] : X0[i] + w].unsqueeze(1)
                nc.vector.tensor_tensor(
                    out=ob,
                    in0=rb.broadcast_to([128, j1 - j0, r1 - r0, w]),
                    in1=src,
                    op=mybir.AluOpType.mult,
                )

            def leaky_act(i, j0, j1, mt, r0=0, r1=BH):
                nc.scalar.activation(
                    out=mt[:, j0:j1, r0:r1],
                    in_=mt[:, j0:j1, r0:r1],
                    func=mybir.ActivationFunctionType.Prelu,
                    alpha=0.1,
                )
                for j in range(j0, j1):
                    store(i, j, mt, r0, r1)

            def dve_ts(i, j0, j1, mt, amt, slot=0):
                # First half of the DVE leaky: am = 0.1*m (4x perf mode).
                # (The Pool/GPSIMD engine rejects ALU tensor ops at the ISA
                # check, so the only elementwise engines are ACT and DVE.)
                nj = j1 - j0
                nc.vector.tensor_scalar(
                    out=amt[:, slot : slot + nj, :, : WI[i]],
                    in0=mt[:, j0:j1],
                    scalar1=0.1,
                    scalar2=None,
                    op0=mybir.AluOpType.mult,
                )

            def dve_max(i, j0, j1, mt, amt, slot=0):
                # Second half: leaky = max(m, am) (2x mode), then stores on
                # the idle Pool/SWDGE queue so they cannot head-of-line
                # block the SP store queue.
                nj = j1 - j0
                nc.vector.tensor_tensor(
                    out=mt[:, j0:j1],
                    in0=mt[:, j0:j1],
                    in1=amt[:, slot : slot + nj, :, : WI[i]],
                    op=mybir.AluOpType.max,
                )
                for j in range(j0, j1):
                    store(i, j, mt, eng=nc.gpsimd)

            def leaky_dve(i, j0, j1, mt, amt):
                dve_ts(i, j0, j1, mt, amt)
                dve_max(i, j0, j1, mt, amt)

            def dve_ts2(i, j0, j1, mt, amt):
                dve_ts(i, j0, j1, mt, amt, slot=2)

            def dve_max2(i, j0, j1, mt, amt):
                dve_max(i, j0, j1, mt, amt, slot=2)

            dummy_t = cpool.tile([128, 4], BF16)

            def dve_dummy():
                # Dependency-free filler op issued between the ts and max
                # of a DVE leaky pair: the list scheduler would otherwise
                # insert the next column's multi-us multiply there (max is
                # not "ready" until ts's semaphore propagates, ~100ns), and
                # the dummy gives that propagation time at ~160ns cost.
                nc.vector.memset(dummy_t[:], 0.0)

            # Leaky split: ACT takes ~7 j's per column, DVE (tensor_scalar
            # + max) the rest, tuned so DVE/ACT/DMA all land near 117us.
            # Chunk boundaries: fine chunks early (short latency -> first
            # stores fire ~8us in), larger chunks in steady state.
            for i in range(D):
                mt = mpool.tile([128, D, BH, WI[i]], BF16)
                amt = apool.tile([128, 3, BH, W], BF16)
                for step in PLAN[i]:
                    kind = step[0]
                    if kind == "d":
                        dve_dummy()
                        continue
                    j0, j1 = step[1], step[2]
                    r0, r1 = (step[3], step[4]) if len(step) > 3 else (0, BH)
                    if kind == "m":
                        mult(i, j0, j1, mt, r0, r1)
                    elif kind == "A":
                        leaky_act(i, j0, j1, mt, r0, r1)
                    elif kind == "t":
                        dve_ts(i, j0, j1, mt, amt)
                    elif kind == "x":
                        dve_max(i, j0, j1, mt, amt)
                    elif kind == "t2":
                        dve_ts2(i, j0, j1, mt, amt)
                    elif kind == "x2":
                        dve_max2(i, j0, j1, mt, amt)
                    else:
                        leaky_dve(i, j0, j1, mt, amt)
    nc.finalize()
    return nc


_cached_nc = None
_last_results = None


def _prep_inputs(ref: np.ndarray, tgt: np.ndarray):
    """ref/tgt: (256, 64, 128) f32 -> per-core blocked/halo'd bf16 arrays.

    Returns ref_blocked (8, 128, 16, 128) and tgt_halo (8, 128, 24, 136),
    partition p = yb*32 + n.
    """
    ref_b = ref.reshape(N_CORES, IPC, YB, BH, W).transpose(0, 2, 1, 3, 4)
    ref_b = np.ascontiguousarray(ref_b).reshape(N_CORES, 128, BH, W)

    tp = np.zeros((IMGS, H + 2 * MD, HALO_W), dtype=np.float32)
    tp[:, MD : MD + H, MD : MD + W] = tgt
    # overlapping 24-row windows starting at yb*16
    idx = (BH * np.arange(YB))[:, None] + np.arange(HALO_H)[None, :]
    halo = tp[:, idx, :]  # (256, 4, 24, 136)
    halo = halo.reshape(N_CORES, IPC, YB, HALO_H, HALO_W).transpose(0, 2, 1, 3, 4)
    halo = np.ascontiguousarray(halo).reshape(N_CORES, 128, HALO_H, HALO_W)
    return ref_b.astype(NP_BF16), halo.astype(NP_BF16)


def kernel(refimg_fea: np.ndarray, targetimg_fea: np.ndarray) -> np.ndarray:
    global _cached_nc, _last_results
    ref = np.asarray(refimg_fea, dtype=np.float32).reshape(IMGS, H, W)
    tgt = np.asarray(targetimg_fea, dtype=np.float32).reshape(IMGS, H, W)
    ref_b, tgt_h = _prep_inputs(ref, tgt)
    if _cached_nc is None:
        _cached_nc = _build()
    nc = _cached_nc
    in_maps = [{"ref": ref_b[k], "tgt": tgt_h[k]} for k in range(N_CORES)]
    res = bass_utils.run_bass_kernel_spmd(nc, in_maps, core_ids=list(range(N_CORES)))
    _last_results = res
    # Per-core output is [yb*32+n, i, j, y_lo, x]; reassemble to
    # [n, i, j, (yb y_lo), x] per core, then stack cores along n.
    parts = []
    for r in res.results:
        flat = np.asarray(r["out"]).astype(np.float32)  # (128, FLAT) packed
        o = np.zeros((128, D, D, BH, W), dtype=np.float32)
        for i in range(D):
            w = WI[i]
            seg = flat[:, BASE[i] : BASE[i] + D * BH * w].reshape(128, D, BH, w)
            o[:, i, :, :, X0[i] : X0[i] + w] = seg
        o = o.reshape(YB, IPC, D, D, BH, W)
        parts.append(o.transpose(1, 2, 3, 0, 4, 5).reshape(IPC, D, D, H, W))
    out = np.concatenate(parts, axis=0)
    return out.reshape(B, C, D, D, H, W)


# revision 35
# speedup vs baseline: 1.8602x; 1.0010x over previous
"""Correlation (9x9 displacement) kernel for Trainium2.

out[b,c,i,j,y,x] = leaky_relu(ref[b,c,y,x] * tgt[b,c, y+j-4, x+i-4], 0.1)
with zero padding outside the target image bounds.

Sharding: the 256 (b,c) images are split 32-per-core across 8 NeuronCores
(pure data parallel, no collectives).

Per-core layout: partition p = yb*32 + n  (yb = row-block 0..3, n = image
0..31).  Each partition stores a halo tile of the target: 24 rows x 136 cols
(its 16-row block plus +-4 halo rows, W plus +-4 pad cols, zeros outside the
image).  Every displacement (i,j) then becomes the plain in-bounds slice
tgt[:, j:j+16, i:i+128], and out-of-bounds zeros compute themselves via
leaky(tgt_pad*0) == 0.

The whole pipeline runs in bf16 (the correctness gate is rel_err < 2e-2;
the bf16 rounding chain measures 1.3e-2 max on these inputs):
 - halves the output-store DMA bytes, which is the hard roofline here
   (the cost model serializes all DMA transfers at 360 B/ns per core), and
 - enables the DVE 2x/4x perf modes for the elementwise work.

Per i-column (fixed width offset i), DVE tensor_tensor computes batches of
j-shifts via a 4D overlapping-window AP on the halo tile (the j and y dims
share the row stride) with a stride-0 broadcast AP for ref.  The leaky
(leaky(v) == max(v, 0.1*v) for slope 0.1) is split across engines so
DVE/ACT/DMA all land near 115.7us:
  7 j's per column -> ACT Prelu (in-place),
  2 j's per column -> DVE tensor_scalar 0.1*m (4x mode) + tensor_tensor
  max (2x mode); their stores dispatch from the idle Pool/SWDGE queue so
  they cannot head-of-line block the SP store queue.
(The Pool/GPSIMD engine rejects ALU tensor ops at the neuronxcc ISA check,
so ACT and DVE are the only elementwise engines.)

Structural zeros are never computed, leaky'd, or stored:
 - zero rows (y+j-4 out of [0,64)) are skipped on the store — they live in
   one 32-partition block (yb=0 for j<4, yb=3 for j>4), so each such store
   splits in two; the DRAM output buffer is pre-zeroed;
 - zero x-columns (x+i-4 out of [0,128)) are packed out entirely: column
   i's tiles and DRAM segment are WI[i] in (124..128) wide.

Column 0 is issued fine-grained (quarter/half-row first chunks, per-j
after) so the first store fires ~7us in instead of ~25us (pipeline fill),
and every column leads with 2-j chunks to keep store release smooth —
production and the 360 B/ns DMA drain are phase-matched within ~1.5us
after warmup.
"""

import ml_dtypes
import numpy as np

import concourse.bacc as bacc
import concourse.mybir as mybir
from concourse import bass_utils
from concourse.ap import AP
from concourse.tile import TileContext

B, C, H, W = 4, 64, 64, 128
MD = 4
D = 2 * MD + 1  # 9
N_CORES = 8
IMGS = B * C  # 256
IPC = IMGS // N_CORES  # 32 images per core
YB = 4  # row blocks per image
BH = H // YB  # 16 rows per block
HALO_H = BH + 2 * MD  # 24
HALO_W = W + 2 * MD  # 136
BF16 = mybir.dt.bfloat16
NP_BF16 = ml_dtypes.bfloat16

# Packed output widths: for width offset i, |i-MD| x-columns are structural
# zeros (out of bounds); neither computed, leaky'd, nor stored.  Column i's
# tiles and DRAM segment are WI[i] wide; the host re-inserts the zeros.
WI = [W - abs(i - MD) for i in range(D)]  # 124..128..124
X0 = [max(0, MD - i) for i in range(D)]  # first valid x per column
_SEG = [D * BH * w for w in WI]
BASE = [sum(_SEG[:i]) for i in range(D)]
FLAT = sum(_SEG)  # 163008 elems per partition


# Per-column op plan for columns 1..8 (column 0 is hand-scheduled fine-
# grained warmup).  Each entry: ("m", j0, j1) DVE multiply chunk,
# ("A", j0, j1) ACT Prelu + stores, ("V", j0, j1) DVE scale+max + stores.
_STD = [
    ("m", 0, 2), ("A", 0, 2), ("m", 2, 4), ("A", 2, 4),
    ("m", 7, 9), ("t", 7, 9), ("d",), ("x", 7, 9),
    ("m", 4, 7), ("A", 4, 5), ("A", 5, 7),
]
_K1 = [
    ("m", 0, 4), ("A", 0, 2), ("A", 2, 4), ("m", 8, 9), ("t", 8, 9), ("d",),
    ("x", 8, 9), ("m", 4, 8), ("A", 4, 6), ("A", 6, 8),
]
_C0 = [
    ("m", 0, 1, 0, 4), ("A", 0, 1, 0, 4), ("m", 0, 1, 4, 8), ("A", 0, 1, 4, 8),
    ("m", 0, 1, 8, 16), ("A", 0, 1, 8, 16),
    ("m", 1, 2), ("A", 1, 2), ("m", 7, 8), ("V", 7, 8),
    ("m", 2, 3), ("A", 2, 3), ("m", 8, 9), ("t2", 8, 9), ("x2", 8, 9),
    ("m", 3, 4), ("A", 3, 4), ("m", 4, 5), ("A", 4, 5), ("m", 5, 7), ("A", 5, 7),
]
# The 8-ACT/1-DVE column sits early (col 3): during warmup ACT-side
# production is the scarcer resource, and the lighter DVE column there
# lets the multiply stream run ahead sooner.
PLAN = {0: _C0, 1: _STD, 2: _STD, 3: _K1, 4: _STD, 5: _STD, 6: _STD, 7: _STD, 8: _STD}


def _build() -> "bacc.Bacc":
    nc = bacc.Bacc(trn_type="TRN2")
    ref_d = nc.dram_tensor("ref", [128, BH, W], BF16, kind="ExternalInput")
    tgt_d = nc.dram_tensor("tgt", [128, HALO_H, HALO_W], BF16, kind="ExternalInput")
    # Partition-major packed output: per partition p = yb*32 + n, column i's
    # segment lives at BASE[i] with row width WI[i]; the host untangles
    # (yb, n) and re-inserts the zero x-borders during unsharding.
    out_d = nc.dram_tensor("out", [128, FLAT], BF16, kind="ExternalOutput")

    with TileContext(nc) as tc:
        with (
            tc.tile_pool(name="const", bufs=1) as cpool,
            tc.tile_pool(name="m", bufs=3) as mpool,
            tc.tile_pool(name="am", bufs=3) as apool,
        ):
            tgt_t = cpool.tile([128, HALO_H, HALO_W], BF16)
            ref_t = cpool.tile([128, BH, W], BF16)
            # Fine-grained loads so the first compute chunk (j=0, rows 0:8)
            # only waits for the first two transfers.  (No finer: HWDGE
            # costs 625ns per DMA, so below ~500B/partition the input
            # loading becomes descriptor-generation-bound.)
            HB = BH // 2  # 8
            nc.sync.dma_start(out=tgt_t[:, :HB], in_=tgt_d[:, :HB])
            nc.sync.dma_start(out=ref_t[:, :HB], in_=ref_d[:, :HB])
            nc.sync.dma_start(out=ref_t[:, HB:], in_=ref_d[:, HB:])
            nc.sync.dma_start(out=tgt_t[:, HB:BH], in_=tgt_d[:, HB:BH])
            nc.sync.dma_start(out=tgt_t[:, BH:], in_=tgt_d[:, BH:])

            ta = tgt_t[:]

            def win(i: int, j0: int, j1: int) -> AP:
                # [p, j, y, x] overlapping-window view of the halo tile:
                # j and y share the row stride.  x starts at the first
                # valid (in-bounds) output column and spans WI[i].
                return AP(
                    tensor=ta.tensor,
                    offset=ta.offset + j0 * HALO_W + i + X0[i],
                    ap=[list(ta.ap[0]), [HALO_W, j1 - j0], [HALO_W, BH], [1, WI[i]]],
                )

            def store(i: int, j: int, mt, r0: int = 0, r1: int = BH, eng=None) -> None:
                # eng: the engine whose queue dispatches the store DMA.
                # Stores must be issued on a queue in data-ready order —
                # a DMA waiting on its data semaphore head-of-line blocks
                # every later DMA on the same queue — so the DVE-leaky
                # path stores go on the idle Pool queue, ACT-path on SP.
                eng = eng or nc.sync
                w = WI[i]
                o0 = BASE[i] + j * BH * w
                dj = j - MD

                def seg(p0, p1, a, b):
                    if a < b:
                        eng.dma_start(
                            out=out_d[p0:p1, o0 + a * w : o0 + b * w],
                            in_=mt[p0:p1, j, a:b],
                        )

                if dj == 0:
                    seg(0, 128, r0, r1)
                elif dj < 0:
                    # rows y < -dj are zero in the yb=0 block (partitions 0:32)
                    seg(IPC, 128, r0, r1)
                    seg(0, IPC, max(r0, -dj), r1)
                else:
                    # rows y >= BH-dj are zero in the yb=3 block (parts 96:128)
                    seg(0, 3 * IPC, r0, r1)
                    seg(3 * IPC, 128, r0, min(r1, BH - dj))

            def mult(i, j0, j1, mt, r0=0, r1=BH):
                # 1-j chunks use a plain slice AP: the manual window AP
                # defeats subtile dependency tracking (conservative
                # whole-tile dep on tgt_t), which matters while the input
                # loads are still in flight.
                w = WI[i]
                if j1 - j0 == 1:
                    c0 = i + X0[i]
                    src = tgt_t[:, j0 + r0 : j0 + r1, c0 : c0 + w].unsqueeze(1)
                    ob = mt[:, j0:j1, r0:r1]
                    rb = ref_t[:, r0:r1, X0[i] : X0[i] + w].unsqueeze(1)
                else:
                    assert (r0, r1) == (0, BH)
                    src = win(i, j0, j1)
                    ob = mt[:, j0:j1]
                    rb = ref_t[:, :, X0[i] : X0[i] + w].unsqueeze(1)
                nc.vector.tensor_tensor(
                    out=ob,
                    in0=rb.broadcast_to([128, j1 - j0, r1 - r0, w]),
                    in1=src,
                    op=mybir.AluOpType.mult,
                )

            def leaky_act(i, j0, j1, mt, r0=0, r1=BH):
                nc.scalar.activation(
                    out=mt[:, j0:j1, r0:r1],
                    in_=mt[:, j0:j1, r0:r1],
                    func=mybir.ActivationFunctionType.Prelu,
                    alpha=0.1,
                )
                for j in range(j0, j1):
                    store(i, j, mt, r0, r1)

            def dve_ts(i, j0, j1, mt, amt, slot=0):
                # First half of the DVE leaky: am = 0.1*m (4x perf mode).
                # (The Pool/GPSIMD engine rejects ALU tensor ops at the ISA
                # check, so the only elementwise engines are ACT and DVE.)
                nj = j1 - j0
                nc.vector.tensor_scalar(
                    out=amt[:, slot : slot + nj, :, : WI[i]],
                    in0=mt[:, j0:j1],
                    scalar1=0.1,
                    scalar2=None,
                    op0=mybir.AluOpType.mult,
                )

            def dve_max(i, j0, j1, mt, amt, slot=0):
                # Second half: leaky = max(m, am) (2x mode), then stores on
                # the idle Pool/SWDGE queue so they cannot head-of-line
                # block the SP store queue.
                nj = j1 - j0
                nc.vector.tensor_tensor(
                    out=mt[:, j0:j1],
                    in0=mt[:, j0:j1],
                    in1=amt[:, slot : slot + nj, :, : WI[i]],
                    op=mybir.AluOpType.max,
                )
                for j in range(j0, j1):
                    store(i, j, mt, eng=nc.gpsimd)

            def leaky_dve(i, j0, j1, mt, amt):
                dve_ts(i, j0, j1, mt, amt)
                dve_max(i, j0, j1, mt, amt)

            def dve_ts2(i, j0, j1, mt, amt):
                dve_ts(i, j0, j1, mt, amt, slot=2)

            def dve_max2(i, j0, j1, mt, amt):
                dve_max(i, j0, j1, mt, amt, slot=2)

            dummy_t = cpool.tile([128, 4], BF16)

            def dve_dummy():
                # Dependency-free filler op issued between the ts and max
                # of a DVE leaky pair: the list scheduler would otherwise
                # insert the next column's multi-us multiply there (max is
                # not "ready" until ts's semaphore propagates, ~100ns), and
                # the dummy gives that propagation time at ~160ns cost.
                nc.vector.memset(dummy_t[:], 0.0)

            # Leaky split: ACT takes ~7 j's per column, DVE (tensor_scalar
            # + max) the rest, tuned so DVE/ACT/DMA all land near 117us.
            # Chunk boundaries: fine chunks early (short latency -> first
            # stores fire ~8us in), larger chunks in steady state.
            for i in range(D):
                mt = mpool.tile([128, D, BH, WI[i]], BF16)
                amt = apool.tile([128, 3, BH, W], BF16)
                for step in PLAN[i]:
                    kind = step[0]
                    if kind == "d":
                        dve_dummy()
                        continue
                    j0, j1 = step[1], step[2]
                    r0, r1 = (step[3], step[4]) if len(step) > 3 else (0, BH)
                    if kind == "m":
                        mult(i, j0, j1, mt, r0, r1)
                    elif kind == "A":
                        leaky_act(i, j0, j1, mt, r0, r1)
                    elif kind == "t":
                        dve_ts(i, j0, j1, mt, amt)
                    elif kind == "x":
                        dve_max(i, j0, j1, mt, amt)
                    elif kind == "t2":
                        dve_ts2(i, j0, j1, mt, amt)
                    elif kind == "x2":
                        dve_max2(i, j0, j1, mt, amt)
                    else:
                        leaky_dve(i, j0, j1, mt, amt)
    nc.finalize()
    return nc


_cached_nc = None
_last_results = None


def _prep_inputs(ref: np.ndarray, tgt: np.ndarray):
    """ref/tgt: (256, 64, 128) f32 -> per-core blocked/halo'd bf16 arrays.

    Returns ref_blocked (8, 128, 16, 128) and tgt_halo (8, 128, 24, 136),
    partition p = yb*32 + n.
    """
    ref_b = ref.reshape(N_CORES, IPC, YB, BH, W).transpose(0, 2, 1, 3, 4)
    ref_b = np.ascontiguousarray(ref_b).reshape(N_CORES, 128, BH, W)

    tp = np.zeros((IMGS, H + 2 * MD, HALO_W), dtype=np.float32)
    tp[:, MD : MD + H, MD : MD + W] = tgt
    # overlapping 24-row windows starting at yb*16
    idx = (BH * np.arange(YB))[:, None] + np.arange(HALO_H)[None, :]
    halo = tp[:, idx, :]  # (256, 4, 24, 136)
    halo = halo.reshape(N_CORES, IPC, YB, HALO_H, HALO_W).transpose(0, 2, 1, 3, 4)
    halo = np.ascontiguousarray(halo).reshape(N_CORES, 128, HALO_H, HALO_W)
    return ref_b.astype(NP_BF16), halo.astype(NP_BF16)


def kernel(refimg_fea: np.ndarray, targetimg_fea: np.ndarray) -> np.ndarray:
    global _cached_nc, _last_results
    ref = np.asarray(refimg_fea, dtype=np.float32).reshape(IMGS, H, W)
    tgt = np.asarray(targetimg_fea, dtype=np.float32).reshape(IMGS, H, W)
    ref_b, tgt_h = _prep_inputs(ref, tgt)
    if _cached_nc is None:
        _cached_nc = _build()
    nc = _cached_nc
    in_maps = [{"ref": ref_b[k], "tgt": tgt_h[k]} for k in range(N_CORES)]
    res = bass_utils.run_bass_kernel_spmd(nc, in_maps, core_ids=list(range(N_CORES)))
    _last_results = res
    # Per-core output is [yb*32+n, i, j, y_lo, x]; reassemble to
    # [n, i, j, (yb y_lo), x] per core, then stack cores along n.
    parts = []
    for r in res.results:
        flat = np.asarray(r["out"]).astype(np.float32)  # (128, FLAT) packed
        o = np.zeros((128, D, D, BH, W), dtype=np.float32)
        for i in range(D):
            w = WI[i]
            seg = flat[:, BASE[i] : BASE[i] + D * BH * w].reshape(128, D, BH, w)
            o[:, i, :, :, X0[i] : X0[i] + w] = seg
        o = o.reshape(YB, IPC, D, D, BH, W)
        parts.append(o.transpose(1, 2, 3, 0, 4, 5).reshape(IPC, D, D, H, W))
    out = np.concatenate(parts, axis=0)
    return out.reshape(B, C, D, D, H, W)
